# revision 34
# baseline (speedup 1.0000x reference)
"""Trainium2 Bass kernel for nn_CrossAttention (dense_transformer).

Strategy: data-parallel over batch B=8 across the 8 NeuronCores (one batch
element per core). Inside each core the phases are software-pipelined so the
ACT-engine exp stream (the serial bottleneck) starts as early as possible
and is never starved:

  - LayerNorm stats via bn_stats/bn_aggr (DVE); the (x - mu) * rstd apply
    runs on GPSIMD (Pool) so the DVE is free for PSUM evacuations.
  - q/kv projections as bf16 PE matmuls with the LN affine and attention
    scale folded into the weights on the host. PSUM evacuation (+bias) on
    the ACT engine during its pre-softmax idle window.
  - Depthwise 3x3 conv on the PE as 9 accumulating matmuls per channel
    chunk with full-128 diagonal weight matrices; emitted between the QK
    and PV blocks of each head pair to fill PE gaps under the exp stream.
  - Attention computed transposed (S^T = k_h^T.T @ q_h^T) so exp runs
    straight out of PSUM on ACT. PV is computed token-major: for each
    query tile, out[128 tq, 65] = P^T(j, tb).T @ v_aug(j) accumulated over
    key tiles j. The free dim is the small head dim (65: v plus a ones
    column for the softmax row-sum), so each accumulation step costs 65
    PE columns instead of 512, and the result lands token-major so no
    output transpose pass is needed.
  - Final normalize+skip-add is one scalar_tensor_tensor per (head, tile)
    reading the PV psum directly; output chunks DMA out per feature chunk.
"""

import numpy as np
import ml_dtypes

import concourse.bass as bass
import concourse.mybir as mybir
import concourse.tile as tile
from concourse import bacc, bass_utils
from concourse.masks import make_identity

F32 = mybir.dt.float32
BF16 = mybir.dt.bfloat16
I16 = mybir.dt.int16
AF = mybir.ActivationFunctionType
OP = mybir.AluOpType

N_CORES = 8
N1 = 1024          # query tokens (= H*W = 32*32)
N2 = 1024          # key tokens
DIM = 512
NH = 8
CH = 64            # head dim
HH = 32            # H
WW = 32            # W
NTOK = N1 // 128   # 8 token tiles
NCH = DIM // 128   # 4 feature chunks
EPS = 1e-5
EXP_A = 128.0 / float(np.log(2.0))      # DVE bitcast-exp: bf16 bits =
EXP_B = 127.0 * 128.0 - 7.45            # round(x*EXP_A + EXP_B) as int16

# tap order: center first so its start=True write covers every pixel of the
# psum accumulation region before the partial (edge-clipped) taps accumulate.
TAPS = [(0, 0), (-1, -1), (-1, 0), (-1, 1), (0, -1), (0, 1),
        (1, -1), (1, 0), (1, 1)]


def _build_program(trace_sim=False, bench_iters=0, phases="ABCDE"):
    nc = bacc.Bacc("TRN2", target_bir_lowering=False, debug=False,
                   enable_asserts=True, num_devices=N_CORES)

    q_ap = nc.dram_tensor("query", [N1, DIM], F32, kind="ExternalInput").ap()
    k_ap = nc.dram_tensor("key", [N2, DIM], F32, kind="ExternalInput").ap()
    wq_ap = nc.dram_tensor("wq", [128, NCH * DIM], BF16, kind="ExternalInput").ap()
    wk_ap = nc.dram_tensor("wk", [128, NCH * DIM], BF16, kind="ExternalInput").ap()
    wv_ap = nc.dram_tensor("wv", [128, NCH * DIM], BF16, kind="ExternalInput").ap()
    dw_ap = nc.dram_tensor("dw", [128, NCH * 9 * 128], BF16, kind="ExternalInput").ap()
    cw_ap = nc.dram_tensor("cw", [128, NCH * 9], F32, kind="ExternalInput").ap()
    b_ap = nc.dram_tensor("b", [128, 3 * NCH], F32, kind="ExternalInput").ap()
    out_ap = nc.dram_tensor("out", [N1, DIM], F32, kind="ExternalOutput").ap()

    with tile.TileContext(nc, trace_sim=trace_sim) as tc:
        if bench_iters:
            with tc.For_i(0, bench_iters, 1):
                _emit(nc, tc, q_ap, k_ap, wq_ap, wk_ap, wv_ap, dw_ap,
                      cw_ap, b_ap, out_ap, phases)
        else:
            _emit(nc, tc, q_ap, k_ap, wq_ap, wk_ap, wv_ap, dw_ap,
                  cw_ap, b_ap, out_ap, phases)
    nc.compile()
    return nc


def _emit(nc, tc, q_ap, k_ap, wq_ap, wk_ap, wv_ap, dw_ap, cw_ap, b_ap,
          out_ap, phases="ABCDE"):
    from contextlib import ExitStack
    ctx = ExitStack()

    const = ctx.enter_context(tc.tile_pool(name="const", bufs=1))

    ident_bf = const.tile([128, 128], BF16, tag="identbf", name="identbf")
    make_identity(nc, ident_bf[:])
    ident_f32 = const.tile([128, 128], F32, tag="identf32", name="identf32")
    make_identity(nc, ident_f32[:])
    eps_t = const.tile([128, 1], F32, tag="eps", name="eps")
    nc.gpsimd.memset(eps_t[:], EPS)

    wq_sb = const.tile([128, NCH * DIM], BF16, tag="wq", name="wq")
    wk_sb = const.tile([128, NCH * DIM], BF16, tag="wk", name="wk")
    wv_sb = const.tile([128, NCH * DIM], BF16, tag="wv", name="wv")
    dw_sb = const.tile([128, NCH * 9 * 128], BF16, tag="dw", name="dw")
    cw_sb = const.tile([128, NCH * 9], F32, tag="cw", name="cw")
    b_sb = const.tile([128, 3 * NCH], F32, tag="b", name="b")

    persist = ctx.enter_context(tc.tile_pool(name="persist", bufs=1))
    lnqT = [persist.tile([128, N1], BF16, tag=f"lnqT{g}", name=f"lnqT{g}") for g in range(NCH)]
    lnkT = [persist.tile([128, N2], BF16, tag=f"lnkT{g}", name=f"lnkT{g}") for g in range(NCH)]
    qT = [persist.tile([128, N1], BF16, tag=f"qT{g}", name=f"qT{g}") for g in range(NCH)]
    kT = [persist.tile([128, N2], BF16, tag=f"kT{g}", name=f"kT{g}") for g in range(NCH)]
    v_aug = [persist.tile([128, NH * (CH + 1)], BF16, tag=f"vaug{i}", name=f"vaug{i}")
             for i in range(NTOK)]
    skipT = [persist.tile([128, N1], BF16, tag=f"skipT{g}", name=f"skipT{g}") for g in range(NCH)]
    PW = WW + 2
    qTp = [persist.tile([128, PW * PW], BF16, tag=f"qTp{g}", name=f"qTp{g}")
           for g in range(NCH)]
    fin_g = [persist.tile([128, NTOK * 128], F32, tag=f"fin{s}", name=f"fin{s}")
             for s in range(2)]
    skb = [persist.tile([128, NTOK * 128], BF16, tag=f"skb{g}", name=f"skb{g}")
           for g in range(NCH)]

    work = ctx.enter_context(tc.tile_pool(name="ln_work", bufs=1))
    big = ctx.enter_context(tc.tile_pool(name="big_psum", bufs=1, space="PSUM"))
    pTp = ctx.enter_context(tc.tile_pool(name="pT_pool", bufs=1))
    finp = ctx.enter_context(tc.tile_pool(name="fin_pool", bufs=1))
    pools = {}

    # ---- Phase A: load + layernorm + transpose --------------------------
    # produces lnT[g] = LN(x)^T  [128 feat, 1024 tok] bf16 per feature chunk
    def phase_a(src_ap, lnT, evac_act=False):
        psA = pools["psA"]
        for half in range(2):
            psts = [psA.tile([128, 1024], BF16, tag=f"pst{s}", bufs=1,
                             name=f"pst{s}") for s in range(2)]
            xt2 = [None, None]
            for pair in range(2):
                # one DMA covers two token tiles: partition p holds rows
                # p and p+128 of the 256-row block
                xt2[pair] = work.tile([128, 2, DIM], F32, tag="xin",
                                      bufs=4, name="xin")
                r0 = (half * 4 + pair * 2) * 128
                nc.sync.dma_start(
                    xt2[pair][:],
                    src_ap[r0:r0 + 256, :].rearrange("(s p) c -> p s c",
                                                     p=128))
            for ii in range(4):
                xt = xt2[ii // 2][:, ii % 2, :]
                bn6 = work.tile([128, 6], F32, tag="bn6", bufs=4, name="bn6")
                nc.vector.bn_stats(out=bn6[:], in_=xt)
                mv = work.tile([128, 2], F32, tag="mv", bufs=4, name="mv")
                nc.vector.bn_aggr(out=mv[:], in_=bn6[:])
                rstd = work.tile([128, 1], F32, tag="rstd", bufs=4, name="rstd")
                nc.scalar.activation(out=rstd[:], in_=mv[:, 1:2],
                                     func=AF.Sqrt, bias=eps_t[:], scale=1.0)
                nc.vector.reciprocal(out=rstd[:], in_=rstd[:])
                ln = work.tile([128, DIM], BF16, tag="ln", bufs=4, name="ln")
                nc.gpsimd.tensor_scalar(
                    out=ln[:], in0=xt,
                    scalar1=mv[:, 0:1], scalar2=rstd[:],
                    op0=OP.subtract, op1=OP.mult)
                for g in range(NCH):
                    nc.tensor.transpose(
                        psts[g // 2][:, (g % 2) * 512 + ii * 128:
                                     (g % 2) * 512 + ii * 128 + 128],
                        ln[:, g * 128:(g + 1) * 128], ident_bf[:])
            for g in range(NCH):
                if evac_act:
                    nc.scalar.copy(
                        out=lnT[g][:, half * 512:(half + 1) * 512],
                        in_=psts[g // 2][:, (g % 2) * 512:(g % 2) * 512 + 512])
                else:
                    nc.vector.tensor_copy(
                        out=lnT[g][:, half * 512:(half + 1) * 512],
                        in_=psts[g // 2][:, (g % 2) * 512:(g % 2) * 512 + 512])

    # ---- Phase B: projections (evac + bias on ACT) ----------------------
    def proj(lnT, w_sb, bcol, dstT, only_m=None, evac_dve=False):
        for m in (range(NCH) if only_m is None else [only_m]):
            ps = pools["st"].tile([128, 1024], F32, tag="st", bufs=2,
                                  name="st")
            for half in range(2):
                for kc in range(NCH):
                    nc.tensor.matmul(
                        ps[:, half * 512:(half + 1) * 512],
                        w_sb[:, kc * DIM + m * 128:kc * DIM + (m + 1) * 128],
                        lnT[kc][:, half * 512:(half + 1) * 512],
                        start=(kc == 0), stop=(kc == NCH - 1))
            if evac_dve:
                nc.vector.tensor_scalar(
                    out=dstT[m][:], in0=ps[:],
                    scalar1=b_sb[:, bcol + m:bcol + m + 1], scalar2=None,
                    op0=OP.add)
            else:
                nc.scalar.activation(
                    out=dstT[m][:], in_=ps[:], func=AF.Identity,
                    bias=b_sb[:, bcol + m:bcol + m + 1], scale=1.0)

    def proj_v(i):
        # ones column for the softmax row-sum trick; data columns are
        # fully overwritten by the evac copy below
        nc.gpsimd.tensor_scalar(
            out=v_aug[i][:].rearrange("p (h c) -> p h c", c=CH + 1)[:, :, CH],
            in0=ident_f32[:, 0:NH], scalar1=0.0, scalar2=1.0,
            op0=OP.mult, op1=OP.add)
        ps = pools["st"].tile([128, 1024], F32, tag="st", bufs=2,
                              name="st")
        for kc in range(NCH):
            nc.tensor.matmul(
                ps[:, 0:512], lnkT[kc][:, i * 128:(i + 1) * 128],
                wv_sb[:, kc * DIM:(kc + 1) * DIM],
                start=(kc == 0), stop=(kc == NCH - 1))
        nc.vector.tensor_copy(
            out=v_aug[i][:].rearrange("p (h c) -> p h c", c=CH + 1)[:, :, 0:CH],
            in_=ps[:, 0:512].rearrange("p (h c) -> p h c", c=CH))

    # ---- Phase C: depthwise conv on PE ----------------------------------
    # skipT[g][c, y*32+x] = sum_taps w(c,tap) * qT[g][c, (y+dy)*32+(x+dx)]
    # via 9 accumulating matmuls per half with full-128 diagonal weight
    # blocks over a zero-padded 34x34 image. Emitted as a generator so the
    # matmuls can be interleaved into the QK stream as PE filler.
    def conv_prep(g):
        qTp3 = qTp[g][:].rearrange("p (y x) -> p y x", x=PW)
        for view, w in ((qTp3[:, 0, :], PW), (qTp3[:, PW - 1, :], PW),
                        (qTp3[:, 1:PW - 1, 0], PW - 2),
                        (qTp3[:, 1:PW - 1, PW - 1], PW - 2)):
            nc.gpsimd.tensor_scalar(
                out=view, in0=ident_f32[:, 0:w], scalar1=0.0,
                scalar2=None, op0=OP.mult)
        nc.vector.tensor_copy(
            out=qTp3[:, 1:HH + 1, 1:WW + 1],
            in_=qT[g][:].rearrange("p (y x) -> p y x", x=WW))

    def conv_taps(g):
        qTp3 = qTp[g][:].rearrange("p (y x) -> p y x", x=PW)
        cps = big.tile([128, 1024], F32, tag="conv", bufs=1, name="conv")
        for yh in range(2):
            for t, (dy, dx) in enumerate(TAPS):
                y0 = yh * 16 + 1 + dy
                nc.tensor.matmul(
                    cps[:, yh * 512:yh * 512 + 512],
                    dw_sb[:, g * 1152 + t * 128:g * 1152 + t * 128 + 128],
                    qTp3[:, y0:y0 + 16, 1 + dx:1 + dx + WW],
                    start=(t == 0), stop=(t == len(TAPS) - 1))
                yield
        nc.vector.tensor_scalar(
            out=skipT[g][:], in0=cps[:], scalar1=b_sb[:, 8 + g:8 + g + 1],
            scalar2=None, op0=OP.add)
        while True:
            yield

    # ---- Phase D: attention -------------------------------------------
    def qk_unit(g, j, pts):
        # S^T for both heads of pair g, key tile j
        for r_i in range(2):
            r = r_i * CH
            st = pools["st"].tile([128, N1], F32, tag="st", bufs=2,
                                  name="st")
            for half in range(2):
                nc.tensor.matmul(
                    st[:, half * 512:(half + 1) * 512],
                    kT[g][r:r + CH, j * 128:(j + 1) * 128],
                    qT[g][r:r + CH, half * 512:(half + 1) * 512],
                    start=True, stop=True, tile_position=(r, 0))
            pt = pTp.tile([128, N1], BF16, tag="pt", bufs=34, name="pt")
            if r_i == 1 and (g == NCH - 1 or j % 4 == 3):
                # DVE bitcast-exp (~2% sawtooth, zero-mean): offloads the
                # ACT engine, which is the serial bottleneck
                nc.vector.tensor_scalar(
                    out=pt[:].bitcast(I16), in0=st[:], scalar1=EXP_A,
                    scalar2=EXP_B, op0=OP.mult, op1=OP.add)
            else:
                nc.scalar.activation(out=pt[:], in_=st[:], func=AF.Exp)
            pts[r_i].append(pt)

    pvstate = {"tile": None, "off": 3}

    def pv_slot():
        # pack 3 PV/sk units per psum tile: 2 ring bufs give a 6-unit-deep
        # pipeline on 2 banks
        if pvstate["off"] == 3:
            pvstate["tile"] = pools["pv"].tile([128, 390], F32, tag="pv",
                                               bufs=2, name="pv")
            pvstate["off"] = 0
        off = pvstate["off"]
        pvstate["off"] += 1
        return pvstate["tile"], off * 130

    def sk_unit(g, tb):
        # transpose one token tile of the conv skip to token-major ahead of
        # time so the PV combine only reads SBUF
        t, off = pv_slot()
        sps = t[:, off:off + 64].bitcast(BF16)
        nc.tensor.transpose(sps, skipT[g][:, tb * 128:(tb + 1) * 128],
                            ident_bf[:])
        nc.vector.tensor_copy(out=skb[g][:, tb * 128:(tb + 1) * 128],
                              in_=sps)

    def pv_unit(g, tb, pts):
        t, off = pv_slot()
        pv = t[:, off:off + 130]
        for r_i in range(2):
            h = 2 * g + r_i
            for j in range(NTOK):
                nc.tensor.matmul(
                    pv[:, r_i * (CH + 1):(r_i + 1) * (CH + 1)],
                    pts[r_i][j][:, tb * 128:(tb + 1) * 128],
                    v_aug[j][:, h * (CH + 1):(h + 1) * (CH + 1)],
                    start=(j == 0), stop=(j == NTOK - 1))
        rc = finp.tile([128, 2], F32, tag="rc", bufs=4, name="rc")
        nc.vector.reciprocal(
            out=rc[:],
            in_=pv.rearrange("p (r c) -> p r c", c=CH + 1)[:, :, CH])
        for r_i in range(2):
            nc.vector.scalar_tensor_tensor(
                out=fin_g[g % 2][:, tb * 128 + r_i * CH:
                                 tb * 128 + (r_i + 1) * CH],
                in0=pv[:, r_i * (CH + 1):r_i * (CH + 1) + CH],
                scalar=rc[:, r_i:r_i + 1],
                in1=skb[g][:, tb * 128 + r_i * CH:tb * 128 + (r_i + 1) * CH],
                op0=OP.mult, op1=OP.add)
        if tb in (3, NTOK - 1):
            lo = 0 if tb == 3 else 4
            nc.sync.dma_start(
                out_ap[lo * 128:(tb + 1) * 128,
                       g * 128:(g + 1) * 128].rearrange(
                    "(t p) c -> p t c", p=128),
                fin_g[g % 2][:, lo * 128:(tb + 1) * 128].rearrange(
                    "p (t c) -> p t c", c=128))

    # ---- emission schedule ----------------------------------------------
    # DMAs are issued on the sync queue in first-need order. Phases are
    # software-pipelined: the conv matmuls and the previous head pair's PV
    # units are interleaved into the QK stream as PE filler so the PE never
    # idles while the ACT engine works through the exp stream.
    nc.sync.dma_start(b_sb[:], b_ap)
    stp = ctx.enter_context(tc.tile_pool(name="st_psum", bufs=1,
                                         space="PSUM"))
    pools["st"] = stp
    with tc.tile_pool(name="ln_psum", bufs=1, space="PSUM") as psA:
        pools["psA"] = psA
        phase_a(q_ap, lnqT)
        nc.sync.dma_start(wq_sb[:], wq_ap)
        phase_a(k_ap, lnkT)
        nc.sync.dma_start(wk_sb[:], wk_ap)
        nc.sync.dma_start(dw_sb[:], dw_ap)
        nc.sync.dma_start(cw_sb[:], cw_ap)
        nc.sync.dma_start(wv_sb[:], wv_ap)
        proj(lnqT, wq_sb, 0, qT, only_m=0)
        proj(lnkT, wk_sb, NCH, kT, only_m=0)

    # Remaining projections, the conv, and the skip transposes are emitted
    # as PE filler units metered into the QK stream: the ACT exp stream is
    # the serial bottleneck, so the PE uses its spare cycles under it.
    with tc.tile_pool(name="pv_psum", bufs=1, space="PSUM") as pvp:
        pools["pv"] = pvp

        def filler_batch(g):
            units = []
            if g + 1 < NCH:
                units.append((2248, lambda m=g + 1: proj(
                    lnqT, wq_sb, 0, qT, only_m=m, evac_dve=True)))
                units.append((2248, lambda m=g + 1: proj(
                    lnkT, wk_sb, NCH, kT, only_m=m, evac_dve=True)))
            if g == 0:
                for i in range(NTOK):
                    units.append((2148, lambda i=i: proj_v(i)))
            units.append((0, lambda: conv_prep(g)))
            taps = conv_taps(g)
            for t in range(9):
                # last unit advances past the final yield so the generator
                # also emits the skipT psum evacuation
                n = 2 if t < 8 else 3
                units.append((n * 512, lambda n=n: [next(taps)
                                                    for _ in range(n)]))
            for tb in range(NTOK):
                units.append((128, lambda tb=tb: sk_unit(g, tb)))
            return units

        pts_all = []
        filler = []
        for g in range(NCH):
            pts = {0: [], 1: []}
            pts_all.append(pts)
            filler.extend(filler_batch(g))
            budget = (sum(c for c, _ in filler) + NTOK - 1) // NTOK
            for j in range(NTOK):
                qk_unit(g, j, pts)
                spent = 0
                while filler and spent < budget:
                    cyc, fn = filler.pop(0)
                    fn()
                    spent += cyc
                if g >= 1:
                    pv_unit(g - 1, j, pts_all[g - 1])
            for cyc, fn in filler:
                fn()
            filler.clear()
        for tb in range(NTOK):
            pv_unit(NCH - 1, tb, pts_all[NCH - 1])

    ctx.close()


_CACHE = {}


def _get_runner():
    """Build the program once and wrap it in a reusable jitted SPMD callable.

    run_bass_kernel_spmd re-traces a fresh closure on every call; caching the
    jitted shard_map keeps steady-state calls at PJRT-execute cost only.
    """
    if "runner" in _CACHE:
        return _CACHE["runner"]

    import jax
    from jax.sharding import Mesh, PartitionSpec
    from jax.experimental.shard_map import shard_map
    from concourse import bass2jax
    import concourse.mybir as mb

    nc = _build_program()
    bass2jax.install_neuronx_cc_hook()

    part_name = (nc.partition_id_tensor.name
                 if nc.partition_id_tensor else None)
    in_names, out_names, out_avals = [], [], []
    for alloc in nc.m.functions[0].allocations:
        if not isinstance(alloc, mb.MemoryLocationSet):
            continue
        name = alloc.memorylocations[0].name
        if alloc.kind == "ExternalInput":
            if name != part_name:
                in_names.append(name)
        elif alloc.kind == "ExternalOutput":
            out_names.append(name)
            out_avals.append(jax.core.ShapedArray(
                tuple(alloc.tensor_shape), mb.dt.np(alloc.dtype)))
    n_params = len(in_names)
    all_names = in_names + out_names
    if part_name is not None:
        all_names = all_names + [part_name]

    def _body(*args):
        operands = list(args)
        if part_name is not None:
            operands.append(bass2jax.partition_id_tensor())
        outs = bass2jax._bass_exec_p.bind(
            *operands,
            out_avals=tuple(out_avals),
            in_names=tuple(all_names),
            out_names=tuple(out_names),
            lowering_input_output_aliases=(),
            sim_require_finite=True,
            sim_require_nnan=True,
            nc=nc,
        )
        return tuple(outs)

    devices = jax.devices()[:N_CORES]
    mesh = Mesh(np.asarray(devices), ("core",))
    n_outs = len(out_names)
    sharded = jax.jit(
        shard_map(_body, mesh=mesh,
                  in_specs=(PartitionSpec("core"),) * (n_params + n_outs),
                  out_specs=(PartitionSpec("core"),) * n_outs,
                  check_rep=False),
        donate_argnums=tuple(range(n_params, n_params + n_outs)),
        keep_unused=True)

    from jax.sharding import NamedSharding
    import jax.numpy as jnp

    zero_shard = NamedSharding(mesh, PartitionSpec("core"))
    make_zeros = jax.jit(
        lambda: tuple(jnp.zeros((N_CORES * a.shape[0], *a.shape[1:]), a.dtype)
                      for a in out_avals),
        out_shardings=(zero_shard,) * len(out_avals))
    dev_cache = {}

    import hashlib

    def run(in_maps):
        concat_in = []
        for name in in_names:
            same = all(in_maps[c][name] is in_maps[0][name]
                       for c in range(N_CORES))
            if same:
                # replicated constants (weights): keep device-resident,
                # keyed by content hash so changed weights re-upload
                key = (name,
                       hashlib.sha1(np.ascontiguousarray(
                           in_maps[0][name]).tobytes()).hexdigest())
                if key not in dev_cache:
                    arr = np.concatenate(
                        [np.asarray(in_maps[c][name])
                         for c in range(N_CORES)], axis=0)
                    dev_cache[key] = jax.device_put(arr, zero_shard)
                concat_in.append(dev_cache[key])
                continue
            concat_in.append(np.concatenate(
                [np.asarray(in_maps[c][name]) for c in range(N_CORES)],
                axis=0))
        out_arrs = sharded(*concat_in, *make_zeros())
        return [
            {name: np.asarray(out_arrs[i]).reshape(
                N_CORES, *out_avals[i].shape)[c]
             for i, name in enumerate(out_names)}
            for c in range(N_CORES)]

    _CACHE["runner"] = run
    return run


def _prepare_in_maps(query, key, gq, bq_ln, gk, bk_ln, Wq, bq, Wkv, bkv,
                     conv_w, conv_b, H, W):
    query = np.asarray(query, np.float32)
    key = np.asarray(key, np.float32)
    gq = np.asarray(gq, np.float32); bq_ln = np.asarray(bq_ln, np.float32)
    gk = np.asarray(gk, np.float32); bk_ln = np.asarray(bk_ln, np.float32)
    Wq = np.asarray(Wq, np.float32); bq = np.asarray(bq, np.float32)
    Wkv = np.asarray(Wkv, np.float32); bkv = np.asarray(bkv, np.float32)
    conv_w = np.asarray(conv_w, np.float32)
    conv_b = np.asarray(conv_b, np.float32)
    assert int(H) == HH and int(W) == WW
    B, n1, dim_q = query.shape
    assert (B, n1, dim_q) == (N_CORES, N1, DIM) and key.shape == (N_CORES, N2, DIM)

    scale = (DIM // NH) ** (-0.5)
    # fold LN affine + attention scale into the q projection; the depthwise
    # conv weights absorb the inverse scale (conv is linear in q).
    wq_pre = (gq[:, None] * Wq) * scale
    bq_pre = (bq_ln @ Wq + bq) * scale
    wkv_pre = gk[:, None] * Wkv
    bkv_pre = bk_ln @ Wkv + bkv
    wk_pre, wv_pre = wkv_pre[:, :DIM], wkv_pre[:, DIM:]
    bk_pre, bv_pre = bkv_pre[:DIM], bkv_pre[DIM:]
    # v-bias: softmax weights sum to 1, so +bv on v == +bv on the output;
    # fold it into the (per-channel) conv bias which is added at the end.
    cb_pre = conv_b + bv_pre

    w8 = conv_w[:, 0, :, :] / scale  # [512, 3, 3]
    # diagonal tap matrices for the PE conv (chunk 0 only) plus plain
    # per-channel tap columns for the Pool-engine conv (chunks 1-3)
    dw = np.zeros((NCH, 9, 128, 128), np.float32)
    cw = np.zeros((128, NCH * 9), np.float32)
    c = np.arange(128)
    for t, (dy, dx) in enumerate(TAPS):
        wt = w8[:, dy + 1, dx + 1].reshape(NCH, 128)
        for g in range(NCH):
            dw[g, t, c, c] = wt[g]
            cw[:, g * 9 + t] = wt[g]
    dw_host = dw.transpose(2, 0, 1, 3).reshape(128, NCH * 9 * 128)

    def wlayout(w):  # [512, 512] -> [128, kc*512 + col]
        return np.ascontiguousarray(
            w.reshape(NCH, 128, DIM).transpose(1, 0, 2).reshape(128, NCH * DIM))

    b_host = np.stack([bq_pre.reshape(NCH, 128), bk_pre.reshape(NCH, 128),
                       cb_pre.reshape(NCH, 128)], 0)  # [3, NCH, 128]
    b_host = np.ascontiguousarray(
        b_host.reshape(3 * NCH, 128).T)  # [128, 12]

    bf = ml_dtypes.bfloat16
    common = {
        "wq": wlayout(wq_pre).astype(bf),
        "wk": wlayout(wk_pre).astype(bf),
        "wv": wlayout(wv_pre).astype(bf),
        "dw": np.ascontiguousarray(dw_host).astype(bf),
        "cw": np.ascontiguousarray(cw),
        "b": b_host,
    }
    return [dict(common, query=np.ascontiguousarray(query[c]),
                 key=np.ascontiguousarray(key[c])) for c in range(N_CORES)]


def kernel(**inputs):
    in_maps = _prepare_in_maps(**inputs)
    run = _get_runner()
    results = run(in_maps)
    return np.stack([results[c]["out"] for c in range(N_CORES)], axis=0)


# revision 35
# speedup vs baseline: 1.0255x; 1.0255x over previous
"""Trainium2 Bass kernel for nn_CrossAttention (dense_transformer).

Strategy: data-parallel over batch B=8 across the 8 NeuronCores (one batch
element per core). Inside each core the phases are software-pipelined so the
ACT-engine exp stream (the serial bottleneck) starts as early as possible
and is never starved:

  - LayerNorm stats via bn_stats/bn_aggr (DVE); the (x - mu) * rstd apply
    runs on GPSIMD (Pool) so the DVE is free for PSUM evacuations.
  - q/kv projections as bf16 PE matmuls with the LN affine and attention
    scale folded into the weights on the host. PSUM evacuation (+bias) on
    the ACT engine during its pre-softmax idle window.
  - Depthwise 3x3 conv on the PE as 9 accumulating matmuls per channel
    chunk with full-128 diagonal weight matrices; emitted between the QK
    and PV blocks of each head pair to fill PE gaps under the exp stream.
  - Attention computed transposed (S^T = k_h^T.T @ q_h^T) so exp runs
    straight out of PSUM on ACT. PV is computed token-major: for each
    query tile, out[128 tq, 65] = P^T(j, tb).T @ v_aug(j) accumulated over
    key tiles j. The free dim is the small head dim (65: v plus a ones
    column for the softmax row-sum), so each accumulation step costs 65
    PE columns instead of 512, and the result lands token-major so no
    output transpose pass is needed.
  - Final normalize+skip-add is one scalar_tensor_tensor per (head, tile)
    reading the PV psum directly; output chunks DMA out per feature chunk.
"""

import numpy as np
import ml_dtypes

import concourse.bass as bass
import concourse.mybir as mybir
import concourse.tile as tile
from concourse import bacc, bass_utils
from concourse.masks import make_identity

F32 = mybir.dt.float32
BF16 = mybir.dt.bfloat16
I16 = mybir.dt.int16
AF = mybir.ActivationFunctionType
OP = mybir.AluOpType

N_CORES = 8
N1 = 1024          # query tokens (= H*W = 32*32)
N2 = 1024          # key tokens
DIM = 512
NH = 8
CH = 64            # head dim
HH = 32            # H
WW = 32            # W
NTOK = N1 // 128   # 8 token tiles
NCH = DIM // 128   # 4 feature chunks
EPS = 1e-5
EXP_A = 128.0 / float(np.log(2.0))      # DVE bitcast-exp: bf16 bits =
EXP_B = 127.0 * 128.0 - 7.45            # round(x*EXP_A + EXP_B) as int16

# tap order: center first so its start=True write covers every pixel of the
# psum accumulation region before the partial (edge-clipped) taps accumulate.
TAPS = [(0, 0), (-1, -1), (-1, 0), (-1, 1), (0, -1), (0, 1),
        (1, -1), (1, 0), (1, 1)]


def _build_program(trace_sim=False, bench_iters=0, phases="ABCDE"):
    nc = bacc.Bacc("TRN2", target_bir_lowering=False, debug=False,
                   enable_asserts=True, num_devices=N_CORES)

    q_ap = nc.dram_tensor("query", [N1, DIM], F32, kind="ExternalInput").ap()
    k_ap = nc.dram_tensor("key", [N2, DIM], F32, kind="ExternalInput").ap()
    wq_ap = nc.dram_tensor("wq", [128, NCH * DIM], BF16, kind="ExternalInput").ap()
    wk_ap = nc.dram_tensor("wk", [128, NCH * DIM], BF16, kind="ExternalInput").ap()
    wv_ap = nc.dram_tensor("wv", [128, NCH * DIM], BF16, kind="ExternalInput").ap()
    dw_ap = nc.dram_tensor("dw", [128, NCH * 9 * 128], BF16, kind="ExternalInput").ap()
    cw_ap = nc.dram_tensor("cw", [128, NCH * 9], F32, kind="ExternalInput").ap()
    b_ap = nc.dram_tensor("b", [128, 3 * NCH], F32, kind="ExternalInput").ap()
    out_ap = nc.dram_tensor("out", [N1, DIM], F32, kind="ExternalOutput").ap()

    with tile.TileContext(nc, trace_sim=trace_sim) as tc:
        if bench_iters:
            with tc.For_i(0, bench_iters, 1):
                _emit(nc, tc, q_ap, k_ap, wq_ap, wk_ap, wv_ap, dw_ap,
                      cw_ap, b_ap, out_ap, phases)
        else:
            _emit(nc, tc, q_ap, k_ap, wq_ap, wk_ap, wv_ap, dw_ap,
                  cw_ap, b_ap, out_ap, phases)
    nc.compile()
    return nc


def _emit(nc, tc, q_ap, k_ap, wq_ap, wk_ap, wv_ap, dw_ap, cw_ap, b_ap,
          out_ap, phases="ABCDE"):
    from contextlib import ExitStack
    ctx = ExitStack()

    const = ctx.enter_context(tc.tile_pool(name="const", bufs=1))

    ident_bf = const.tile([128, 128], BF16, tag="identbf", name="identbf")
    make_identity(nc, ident_bf[:])
    ident_f32 = const.tile([128, 128], F32, tag="identf32", name="identf32")
    make_identity(nc, ident_f32[:])
    eps_t = const.tile([128, 1], F32, tag="eps", name="eps")
    nc.gpsimd.memset(eps_t[:], EPS)

    wq_sb = const.tile([128, NCH * DIM], BF16, tag="wq", name="wq")
    wk_sb = const.tile([128, NCH * DIM], BF16, tag="wk", name="wk")
    wv_sb = const.tile([128, NCH * DIM], BF16, tag="wv", name="wv")
    dw_sb = const.tile([128, NCH * 9 * 128], BF16, tag="dw", name="dw")
    cw_sb = const.tile([128, NCH * 9], F32, tag="cw", name="cw")
    b_sb = const.tile([128, 3 * NCH], F32, tag="b", name="b")

    persist = ctx.enter_context(tc.tile_pool(name="persist", bufs=1))
    lnqT = [persist.tile([128, N1], BF16, tag=f"lnqT{g}", name=f"lnqT{g}") for g in range(NCH)]
    lnkT = [persist.tile([128, N2], BF16, tag=f"lnkT{g}", name=f"lnkT{g}") for g in range(NCH)]
    qT = [persist.tile([128, N1], BF16, tag=f"qT{g}", name=f"qT{g}") for g in range(NCH)]
    kT = [persist.tile([128, N2], BF16, tag=f"kT{g}", name=f"kT{g}") for g in range(NCH)]
    v_aug = [persist.tile([128, NH * (CH + 1)], BF16, tag=f"vaug{i}", name=f"vaug{i}")
             for i in range(NTOK)]
    skipT = [persist.tile([128, N1], BF16, tag=f"skipT{g}", name=f"skipT{g}") for g in range(NCH)]
    PW = WW + 2
    qTp = [persist.tile([128, PW * PW], BF16, tag=f"qTp{g}", name=f"qTp{g}")
           for g in range(NCH)]
    fin_g = [persist.tile([128, NTOK * 128], F32, tag=f"fin{s}", name=f"fin{s}")
             for s in range(2)]
    skb = [persist.tile([128, NTOK * 128], BF16, tag=f"skb{g}", name=f"skb{g}")
           for g in range(NCH)]

    work = ctx.enter_context(tc.tile_pool(name="ln_work", bufs=1))
    big = ctx.enter_context(tc.tile_pool(name="big_psum", bufs=1, space="PSUM"))
    pTp = ctx.enter_context(tc.tile_pool(name="pT_pool", bufs=1))
    finp = ctx.enter_context(tc.tile_pool(name="fin_pool", bufs=1))
    pools = {}

    # ---- Phase A: load + layernorm + transpose --------------------------
    # produces lnT[g] = LN(x)^T  [128 feat, 1024 tok] bf16 per feature chunk
    def phase_a(src_ap, lnT, evac_act=False):
        psA = pools["psA"]
        for half in range(2):
            psts = [psA.tile([128, 1024], BF16, tag=f"pst{s}", bufs=1,
                             name=f"pst{s}") for s in range(2)]
            xt2 = [None, None]
            for pair in range(2):
                # one DMA covers two token tiles: partition p holds rows
                # p and p+128 of the 256-row block
                xt2[pair] = work.tile([128, 2, DIM], F32, tag="xin",
                                      bufs=4, name="xin")
                r0 = (half * 4 + pair * 2) * 128
                nc.sync.dma_start(
                    xt2[pair][:],
                    src_ap[r0:r0 + 256, :].rearrange("(s p) c -> p s c",
                                                     p=128))
            for ii in range(4):
                xt = xt2[ii // 2][:, ii % 2, :]
                bn6 = work.tile([128, 6], F32, tag="bn6", bufs=4, name="bn6")
                nc.vector.bn_stats(out=bn6[:], in_=xt)
                mv = work.tile([128, 2], F32, tag="mv", bufs=4, name="mv")
                nc.vector.bn_aggr(out=mv[:], in_=bn6[:])
                rstd = work.tile([128, 1], F32, tag="rstd", bufs=4, name="rstd")
                nc.scalar.activation(out=rstd[:], in_=mv[:, 1:2],
                                     func=AF.Sqrt, bias=eps_t[:], scale=1.0)
                nc.vector.reciprocal(out=rstd[:], in_=rstd[:])
                ln = work.tile([128, DIM], BF16, tag="ln", bufs=4, name="ln")
                nc.gpsimd.tensor_scalar(
                    out=ln[:], in0=xt,
                    scalar1=mv[:, 0:1], scalar2=rstd[:],
                    op0=OP.subtract, op1=OP.mult)
                for g in range(NCH):
                    nc.tensor.transpose(
                        psts[g // 2][:, (g % 2) * 512 + ii * 128:
                                     (g % 2) * 512 + ii * 128 + 128],
                        ln[:, g * 128:(g + 1) * 128], ident_bf[:])
            for g in range(NCH):
                if evac_act:
                    nc.scalar.copy(
                        out=lnT[g][:, half * 512:(half + 1) * 512],
                        in_=psts[g // 2][:, (g % 2) * 512:(g % 2) * 512 + 512])
                else:
                    nc.vector.tensor_copy(
                        out=lnT[g][:, half * 512:(half + 1) * 512],
                        in_=psts[g // 2][:, (g % 2) * 512:(g % 2) * 512 + 512])

    # ---- Phase B: projections (evac + bias on ACT) ----------------------
    def proj(lnT, w_sb, bcol, dstT, only_m=None, evac_dve=False):
        for m in (range(NCH) if only_m is None else [only_m]):
            ps = pools["st"].tile([128, 1024], F32, tag="st", bufs=2,
                                  name="st")
            for half in range(2):
                for kc in range(NCH):
                    nc.tensor.matmul(
                        ps[:, half * 512:(half + 1) * 512],
                        w_sb[:, kc * DIM + m * 128:kc * DIM + (m + 1) * 128],
                        lnT[kc][:, half * 512:(half + 1) * 512],
                        start=(kc == 0), stop=(kc == NCH - 1))
            if evac_dve:
                nc.vector.tensor_scalar(
                    out=dstT[m][:], in0=ps[:],
                    scalar1=b_sb[:, bcol + m:bcol + m + 1], scalar2=None,
                    op0=OP.add)
            else:
                nc.scalar.activation(
                    out=dstT[m][:], in_=ps[:], func=AF.Identity,
                    bias=b_sb[:, bcol + m:bcol + m + 1], scale=1.0)

    def proj_v(i):
        # ones column for the softmax row-sum trick; data columns are
        # fully overwritten by the evac copy below
        nc.gpsimd.tensor_scalar(
            out=v_aug[i][:].rearrange("p (h c) -> p h c", c=CH + 1)[:, :, CH],
            in0=ident_f32[:, 0:NH], scalar1=0.0, scalar2=1.0,
            op0=OP.mult, op1=OP.add)
        ps = pools["st"].tile([128, 1024], F32, tag="st", bufs=2,
                              name="st")
        for kc in range(NCH):
            nc.tensor.matmul(
                ps[:, 0:512], lnkT[kc][:, i * 128:(i + 1) * 128],
                wv_sb[:, kc * DIM:(kc + 1) * DIM],
                start=(kc == 0), stop=(kc == NCH - 1))
        nc.vector.tensor_copy(
            out=v_aug[i][:].rearrange("p (h c) -> p h c", c=CH + 1)[:, :, 0:CH],
            in_=ps[:, 0:512].rearrange("p (h c) -> p h c", c=CH))

    # ---- Phase C: depthwise conv on PE ----------------------------------
    # skipT[g][c, y*32+x] = sum_taps w(c,tap) * qT[g][c, (y+dy)*32+(x+dx)]
    # via 9 accumulating matmuls per half with full-128 diagonal weight
    # blocks over a zero-padded 34x34 image. Emitted as a generator so the
    # matmuls can be interleaved into the QK stream as PE filler.
    def conv_prep(g):
        qTp3 = qTp[g][:].rearrange("p (y x) -> p y x", x=PW)
        for view, w in ((qTp3[:, 0, :], PW), (qTp3[:, PW - 1, :], PW),
                        (qTp3[:, 1:PW - 1, 0], PW - 2),
                        (qTp3[:, 1:PW - 1, PW - 1], PW - 2)):
            nc.gpsimd.tensor_scalar(
                out=view, in0=ident_f32[:, 0:w], scalar1=0.0,
                scalar2=None, op0=OP.mult)
        nc.vector.tensor_copy(
            out=qTp3[:, 1:HH + 1, 1:WW + 1],
            in_=qT[g][:].rearrange("p (y x) -> p y x", x=WW))

    def conv_taps(g):
        qTp3 = qTp[g][:].rearrange("p (y x) -> p y x", x=PW)
        cps = big.tile([128, 1024], F32, tag="conv", bufs=1, name="conv")
        for yh in range(2):
            for t, (dy, dx) in enumerate(TAPS):
                y0 = yh * 16 + 1 + dy
                nc.tensor.matmul(
                    cps[:, yh * 512:yh * 512 + 512],
                    dw_sb[:, g * 1152 + t * 128:g * 1152 + t * 128 + 128],
                    qTp3[:, y0:y0 + 16, 1 + dx:1 + dx + WW],
                    start=(t == 0), stop=(t == len(TAPS) - 1))
                yield
        nc.vector.tensor_scalar(
            out=skipT[g][:], in0=cps[:], scalar1=b_sb[:, 8 + g:8 + g + 1],
            scalar2=None, op0=OP.add)
        while True:
            yield

    # ---- Phase D: attention -------------------------------------------
    def qk_unit(g, j, pts):
        # S^T for both heads of pair g, key tile j
        for r_i in range(2):
            r = r_i * CH
            st = pools["st"].tile([128, N1], F32, tag="st", bufs=2,
                                  name="st")
            for half in range(2):
                nc.tensor.matmul(
                    st[:, half * 512:(half + 1) * 512],
                    kT[g][r:r + CH, j * 128:(j + 1) * 128],
                    qT[g][r:r + CH, half * 512:(half + 1) * 512],
                    start=True, stop=True, tile_position=(r, 0))
            pt = pTp.tile([128, N1], BF16, tag="pt", bufs=34, name="pt")
            if r_i == 1 and (j % 2 == 1 if g == NCH - 1 else j % 4 == 3):
                # DVE bitcast-exp (~2% sawtooth, zero-mean): offloads the
                # ACT engine, which is the serial bottleneck
                nc.vector.tensor_scalar(
                    out=pt[:].bitcast(I16), in0=st[:], scalar1=EXP_A,
                    scalar2=EXP_B, op0=OP.mult, op1=OP.add)
            else:
                nc.scalar.activation(out=pt[:], in_=st[:], func=AF.Exp)
            pts[r_i].append(pt)

    pvstate = {"tile": None, "off": 3}

    def pv_slot():
        # pack 3 PV/sk units per psum tile: 2 ring bufs give a 6-unit-deep
        # pipeline on 2 banks
        if pvstate["off"] == 3:
            pvstate["tile"] = pools["pv"].tile([128, 390], F32, tag="pv",
                                               bufs=2, name="pv")
            pvstate["off"] = 0
        off = pvstate["off"]
        pvstate["off"] += 1
        return pvstate["tile"], off * 130

    def sk_unit(g, tb):
        # transpose one token tile of the conv skip to token-major ahead of
        # time so the PV combine only reads SBUF
        t, off = pv_slot()
        sps = t[:, off:off + 64].bitcast(BF16)
        nc.tensor.transpose(sps, skipT[g][:, tb * 128:(tb + 1) * 128],
                            ident_bf[:])
        nc.vector.tensor_copy(out=skb[g][:, tb * 128:(tb + 1) * 128],
                              in_=sps)

    def pv_unit(g, tb, pts):
        t, off = pv_slot()
        pv = t[:, off:off + 130]
        for r_i in range(2):
            h = 2 * g + r_i
            for j in range(NTOK):
                nc.tensor.matmul(
                    pv[:, r_i * (CH + 1):(r_i + 1) * (CH + 1)],
                    pts[r_i][j][:, tb * 128:(tb + 1) * 128],
                    v_aug[j][:, h * (CH + 1):(h + 1) * (CH + 1)],
                    start=(j == 0), stop=(j == NTOK - 1))
        rc = finp.tile([128, 2], F32, tag="rc", bufs=4, name="rc")
        nc.vector.reciprocal(
            out=rc[:],
            in_=pv.rearrange("p (r c) -> p r c", c=CH + 1)[:, :, CH])
        for r_i in range(2):
            nc.vector.scalar_tensor_tensor(
                out=fin_g[g % 2][:, tb * 128 + r_i * CH:
                                 tb * 128 + (r_i + 1) * CH],
                in0=pv[:, r_i * (CH + 1):r_i * (CH + 1) + CH],
                scalar=rc[:, r_i:r_i + 1],
                in1=skb[g][:, tb * 128 + r_i * CH:tb * 128 + (r_i + 1) * CH],
                op0=OP.mult, op1=OP.add)
        if tb in (3, NTOK - 1):
            lo = 0 if tb == 3 else 4
            nc.sync.dma_start(
                out_ap[lo * 128:(tb + 1) * 128,
                       g * 128:(g + 1) * 128].rearrange(
                    "(t p) c -> p t c", p=128),
                fin_g[g % 2][:, lo * 128:(tb + 1) * 128].rearrange(
                    "p (t c) -> p t c", c=128))

    # ---- emission schedule ----------------------------------------------
    # DMAs are issued on the sync queue in first-need order. Phases are
    # software-pipelined: the conv matmuls and the previous head pair's PV
    # units are interleaved into the QK stream as PE filler so the PE never
    # idles while the ACT engine works through the exp stream.
    nc.sync.dma_start(b_sb[:], b_ap)
    stp = ctx.enter_context(tc.tile_pool(name="st_psum", bufs=1,
                                         space="PSUM"))
    pools["st"] = stp
    with tc.tile_pool(name="ln_psum", bufs=1, space="PSUM") as psA:
        pools["psA"] = psA
        phase_a(q_ap, lnqT)
        nc.sync.dma_start(wq_sb[:], wq_ap)
        phase_a(k_ap, lnkT)
        nc.sync.dma_start(wk_sb[:], wk_ap)
        nc.sync.dma_start(dw_sb[:], dw_ap)
        nc.sync.dma_start(cw_sb[:], cw_ap)
        nc.sync.dma_start(wv_sb[:], wv_ap)
        proj(lnqT, wq_sb, 0, qT, only_m=0)
        proj(lnkT, wk_sb, NCH, kT, only_m=0)

    # Remaining projections, the conv, and the skip transposes are emitted
    # as PE filler units metered into the QK stream: the ACT exp stream is
    # the serial bottleneck, so the PE uses its spare cycles under it.
    with tc.tile_pool(name="pv_psum", bufs=1, space="PSUM") as pvp:
        pools["pv"] = pvp

        def filler_batch(g):
            units = []
            if g + 1 < NCH:
                units.append((2248, lambda m=g + 1: proj(
                    lnqT, wq_sb, 0, qT, only_m=m, evac_dve=True)))
                units.append((2248, lambda m=g + 1: proj(
                    lnkT, wk_sb, NCH, kT, only_m=m, evac_dve=True)))
            if g == 0:
                for i in range(NTOK):
                    units.append((2148, lambda i=i: proj_v(i)))
            units.append((0, lambda: conv_prep(g)))
            taps = conv_taps(g)
            for t in range(9):
                # last unit advances past the final yield so the generator
                # also emits the skipT psum evacuation
                n = 2 if t < 8 else 3
                units.append((n * 512, lambda n=n: [next(taps)
                                                    for _ in range(n)]))
            for tb in range(NTOK):
                units.append((128, lambda tb=tb: sk_unit(g, tb)))
            return units

        pts_all = []
        filler = []
        for g in range(NCH):
            pts = {0: [], 1: []}
            pts_all.append(pts)
            filler.extend(filler_batch(g))
            budget = (sum(c for c, _ in filler) + NTOK - 1) // NTOK
            for j in range(NTOK):
                qk_unit(g, j, pts)
                spent = 0
                while filler and spent < budget:
                    cyc, fn = filler.pop(0)
                    fn()
                    spent += cyc
                if g >= 1:
                    pv_unit(g - 1, j, pts_all[g - 1])
            for cyc, fn in filler:
                fn()
            filler.clear()
        for tb in range(NTOK):
            pv_unit(NCH - 1, tb, pts_all[NCH - 1])

    ctx.close()


_CACHE = {}


def _get_runner():
    """Build the program once and wrap it in a reusable jitted SPMD callable.

    run_bass_kernel_spmd re-traces a fresh closure on every call; caching the
    jitted shard_map keeps steady-state calls at PJRT-execute cost only.
    """
    if "runner" in _CACHE:
        return _CACHE["runner"]

    import jax
    from jax.sharding import Mesh, PartitionSpec
    from jax.experimental.shard_map import shard_map
    from concourse import bass2jax
    import concourse.mybir as mb

    nc = _build_program()
    bass2jax.install_neuronx_cc_hook()

    part_name = (nc.partition_id_tensor.name
                 if nc.partition_id_tensor else None)
    in_names, out_names, out_avals = [], [], []
    for alloc in nc.m.functions[0].allocations:
        if not isinstance(alloc, mb.MemoryLocationSet):
            continue
        name = alloc.memorylocations[0].name
        if alloc.kind == "ExternalInput":
            if name != part_name:
                in_names.append(name)
        elif alloc.kind == "ExternalOutput":
            out_names.append(name)
            out_avals.append(jax.core.ShapedArray(
                tuple(alloc.tensor_shape), mb.dt.np(alloc.dtype)))
    n_params = len(in_names)
    all_names = in_names + out_names
    if part_name is not None:
        all_names = all_names + [part_name]

    def _body(*args):
        operands = list(args)
        if part_name is not None:
            operands.append(bass2jax.partition_id_tensor())
        outs = bass2jax._bass_exec_p.bind(
            *operands,
            out_avals=tuple(out_avals),
            in_names=tuple(all_names),
            out_names=tuple(out_names),
            lowering_input_output_aliases=(),
            sim_require_finite=True,
            sim_require_nnan=True,
            nc=nc,
        )
        return tuple(outs)

    devices = jax.devices()[:N_CORES]
    mesh = Mesh(np.asarray(devices), ("core",))
    n_outs = len(out_names)
    sharded = jax.jit(
        shard_map(_body, mesh=mesh,
                  in_specs=(PartitionSpec("core"),) * (n_params + n_outs),
                  out_specs=(PartitionSpec("core"),) * n_outs,
                  check_rep=False),
        donate_argnums=tuple(range(n_params, n_params + n_outs)),
        keep_unused=True)

    from jax.sharding import NamedSharding
    import jax.numpy as jnp

    zero_shard = NamedSharding(mesh, PartitionSpec("core"))
    make_zeros = jax.jit(
        lambda: tuple(jnp.zeros((N_CORES * a.shape[0], *a.shape[1:]), a.dtype)
                      for a in out_avals),
        out_shardings=(zero_shard,) * len(out_avals))
    dev_cache = {}

    import hashlib

    def run(in_maps):
        concat_in = []
        for name in in_names:
            same = all(in_maps[c][name] is in_maps[0][name]
                       for c in range(N_CORES))
            if same:
                # replicated constants (weights): keep device-resident,
                # keyed by content hash so changed weights re-upload
                key = (name,
                       hashlib.sha1(np.ascontiguousarray(
                           in_maps[0][name]).tobytes()).hexdigest())
                if key not in dev_cache:
                    arr = np.concatenate(
                        [np.asarray(in_maps[c][name])
                         for c in range(N_CORES)], axis=0)
                    dev_cache[key] = jax.device_put(arr, zero_shard)
                concat_in.append(dev_cache[key])
                continue
            concat_in.append(np.concatenate(
                [np.asarray(in_maps[c][name]) for c in range(N_CORES)],
                axis=0))
        out_arrs = sharded(*concat_in, *make_zeros())
        return [
            {name: np.asarray(out_arrs[i]).reshape(
                N_CORES, *out_avals[i].shape)[c]
             for i, name in enumerate(out_names)}
            for c in range(N_CORES)]

    _CACHE["runner"] = run
    return run


def _prepare_in_maps(query, key, gq, bq_ln, gk, bk_ln, Wq, bq, Wkv, bkv,
                     conv_w, conv_b, H, W):
    query = np.asarray(query, np.float32)
    key = np.asarray(key, np.float32)
    gq = np.asarray(gq, np.float32); bq_ln = np.asarray(bq_ln, np.float32)
    gk = np.asarray(gk, np.float32); bk_ln = np.asarray(bk_ln, np.float32)
    Wq = np.asarray(Wq, np.float32); bq = np.asarray(bq, np.float32)
    Wkv = np.asarray(Wkv, np.float32); bkv = np.asarray(bkv, np.float32)
    conv_w = np.asarray(conv_w, np.float32)
    conv_b = np.asarray(conv_b, np.float32)
    assert int(H) == HH and int(W) == WW
    B, n1, dim_q = query.shape
    assert (B, n1, dim_q) == (N_CORES, N1, DIM) and key.shape == (N_CORES, N2, DIM)

    scale = (DIM // NH) ** (-0.5)
    # fold LN affine + attention scale into the q projection; the depthwise
    # conv weights absorb the inverse scale (conv is linear in q).
    wq_pre = (gq[:, None] * Wq) * scale
    bq_pre = (bq_ln @ Wq + bq) * scale
    wkv_pre = gk[:, None] * Wkv
    bkv_pre = bk_ln @ Wkv + bkv
    wk_pre, wv_pre = wkv_pre[:, :DIM], wkv_pre[:, DIM:]
    bk_pre, bv_pre = bkv_pre[:DIM], bkv_pre[DIM:]
    # v-bias: softmax weights sum to 1, so +bv on v == +bv on the output;
    # fold it into the (per-channel) conv bias which is added at the end.
    cb_pre = conv_b + bv_pre

    w8 = conv_w[:, 0, :, :] / scale  # [512, 3, 3]
    # diagonal tap matrices for the PE conv (chunk 0 only) plus plain
    # per-channel tap columns for the Pool-engine conv (chunks 1-3)
    dw = np.zeros((NCH, 9, 128, 128), np.float32)
    cw = np.zeros((128, NCH * 9), np.float32)
    c = np.arange(128)
    for t, (dy, dx) in enumerate(TAPS):
        wt = w8[:, dy + 1, dx + 1].reshape(NCH, 128)
        for g in range(NCH):
            dw[g, t, c, c] = wt[g]
            cw[:, g * 9 + t] = wt[g]
    dw_host = dw.transpose(2, 0, 1, 3).reshape(128, NCH * 9 * 128)

    def wlayout(w):  # [512, 512] -> [128, kc*512 + col]
        return np.ascontiguousarray(
            w.reshape(NCH, 128, DIM).transpose(1, 0, 2).reshape(128, NCH * DIM))

    b_host = np.stack([bq_pre.reshape(NCH, 128), bk_pre.reshape(NCH, 128),
                       cb_pre.reshape(NCH, 128)], 0)  # [3, NCH, 128]
    b_host = np.ascontiguousarray(
        b_host.reshape(3 * NCH, 128).T)  # [128, 12]

    bf = ml_dtypes.bfloat16
    common = {
        "wq": wlayout(wq_pre).astype(bf),
        "wk": wlayout(wk_pre).astype(bf),
        "wv": wlayout(wv_pre).astype(bf),
        "dw": np.ascontiguousarray(dw_host).astype(bf),
        "cw": np.ascontiguousarray(cw),
        "b": b_host,
    }
    return [dict(common, query=np.ascontiguousarray(query[c]),
                 key=np.ascontiguousarray(key[c])) for c in range(N_CORES)]


def kernel(**inputs):
    in_maps = _prepare_in_maps(**inputs)
    run = _get_runner()
    results = run(in_maps)
    return np.stack([results[c]["out"] for c in range(N_CORES)], axis=0)
